# revision 6
# baseline (speedup 1.0000x reference)
"""Trainium2 Bass kernel v2: pool -> AllGather(features) -> N-sharded linear
-> AllToAll(per-query results) -> per-sample topk+gather.

Stage 1 (data parallel): each core pools its 8 samples' images -> [8, 1200]
features, written straight into the AllGather input buffer in DRAM.
AllGather replicates all 64 samples' features to every core (307KB).

Stage 2 (model parallel): W's output dim is padded 300->336 queries and
sharded 8 ways (42 queries = 3528 cols/core, 16.9MB instead of 121MB).
Each core computes y[64, 3528] and reduces it to per-query
[boxes(4), score, argmax-id] records.

Stage 3: AllToAll (64KB) routes each sample's 8 query-shard records to the
sample's owner core; owner runs top-150 + gather exactly like the
data-parallel baseline and writes its 8 samples' output.

Self-contained: hardcodes all shapes; builds one SPMD Bass program and runs
it via run_bass_kernel_spmd on cores 0-7.
"""

import os
import sys
from contextlib import ExitStack

import numpy as np

for _p in ("/opt/trn_rl_repo", "/root/.axon_site/_ro/trn_rl_repo"):
    if os.path.isdir(_p) and _p not in sys.path:
        sys.path.append(_p)

import concourse.bass as bass
import concourse.tile as tile
from concourse import bacc, library_config, mybir
from concourse.bass_utils import run_bass_kernel_spmd

dt = mybir.dt
F32 = dt.float32
AX = mybir.AxisListType
OP = mybir.AluOpType

# ---------------- problem constants (hardcoded) ----------------
B, CHN, HIMG, WIMG = 64, 3, 640, 640
NQ, NCHAN, NCL, TOPK = 300, 84, 80, 150
KDIM = 1200                         # 3*20*20 features
NCORES = 8
BPC = B // NCORES                   # samples per core = 8
KT, KTS = 10, 120                   # GEMM K tiling: 10 x 120
NQP = 336                           # queries padded to 8*42
QSH = NQP // NCORES                 # queries per core shard = 42
NSH = QSH * NCHAN                   # stage-2 cols per core = 3528
QPC = 6                             # queries per psum chunk
NCHUNK = QPC * NCHAN                # 504 cols per chunk (psum bank)
NCH2 = NSH // NCHUNK                # 7 chunks
PQ = QSH * 7                        # a2a payload: [42 x (box4,score,id)] + [42 scores]
SCALE = np.float64(1.0) / (32 * 32 * 255)
NEG = -3.0e38
PADVAL = -1.0                       # padded W cols: score ~ -560, never top-k
NIDX = 160                          # padded top-k index count (152 used)
NROUND = 19                         # 19 rounds x 8 = 152 >= 150


def build_program():
    nc = bacc.Bacc("TRN2", target_bir_lowering=False, debug=False,
                   num_devices=NCORES)
    # x host-packed uint8, partition-major: every DMA is a linear read
    x_d = nc.dram_tensor(
        "x", [BPC, CHN, 128, 5, WIMG], dt.uint8, kind="ExternalInput"
    )
    # per-core W shard, (k-tile, col-chunk) major so each chunk is linear
    w_d = nc.dram_tensor("w", [KT, 3, KTS, NSH // 3], F32, kind="ExternalInput")
    g4_d = nc.dram_tensor("g4", [128, 4], F32, kind="ExternalInput")
    id64_d = nc.dram_tensor("id64", [B, B], F32, kind="ExternalInput")
    iod_d = nc.dram_tensor("iod", [128, NCL], F32, kind="ExternalInput")
    out_d = nc.dram_tensor("out", [BPC, TOPK, 6], F32, kind="ExternalOutput")

    with tile.TileContext(nc) as tc:
        with ExitStack() as ctx:
            _body(ctx, tc, x_d, w_d, g4_d, id64_d, iod_d, out_d)
    nc.finalize()
    return nc


def _body(ctx, tc, x_d, w_d, g4_d, id64_d, iod_d, out_d):
    nc = tc.nc

    # ---------------- persistent tiles ----------------
    P = ctx.enter_context(tc.tile_pool(name="persist", bufs=1))
    D = ctx.enter_context(tc.tile_pool(name="dramp", bufs=1, space="DRAM"))

    g4 = P.tile([128, 4], F32, tag="g4")
    id64 = P.tile([B, B], F32, tag="id64")
    iod = P.tile([128, NCL], F32, tag="iod")

    # W shard: all 10 k-tiles resident (16.9MB SBUF). The chunk DMAs are
    # issued after the pooling loop, gated behind the last x transfer, so
    # the x feed owns the DMA rings while pooling runs.
    wt = [P.tile([KTS, NSH], F32, tag=f"wt{k}", name=f"wt{k}") for k in range(KT)]
    CW = NSH // 3

    # a tiny dummy AllReduce fired at t~0: it rendezvouses the 8 cores on
    # the CC cores while pooling runs, so the launch skew (~17us) is paid
    # here -- hidden under pooling -- instead of at the feature exchange.
    sk = D.tile([8, 4], F32, tag="sk")
    nc.gpsimd.collective_compute(
        "AllReduce",
        mybir.AluOpType.add,
        replica_groups=[list(range(NCORES))],
        ins=[sk[:].opt()],
        outs=[sk[:].opt()],
    )

    # collective bounce buffers (DRAM). The feature gather runs as an
    # AllToAll with the local block replicated 8x on the send side: block j
    # holds MY pooled rows, so core c's output block j = core j's rows --
    # the full gather -- while dodging the mesh-AllGather's ring latency.
    agin = D.tile([B, KDIM], F32, tag="agin")
    agout = D.tile([B, KDIM], F32, tag="agout")
    a2ain = D.tile([B, PQ], F32, tag="a2ain")       # records + score trailer
    a2aout = D.tile([B, PQ], F32, tag="a2aout")
    tsc = D.tile([BPC, NIDX], dt.int16, tag="tsc")  # topk idx bounce

    yA = P.tile([B, KDIM], F32, tag="yA")           # gathered features SBUF
    ptA = [P.tile([KTS, B], F32, tag=f"pt{k}", name=f"pt{k}") for k in range(KT)]
    a2as = P.tile([B, PQ], F32, tag="a2as")         # a2a send staging
    mx = P.tile([B, QPC], F32, tag="mx")
    eq = P.tile([B, QPC * NCL], F32, tag="eq")
    am = P.tile([B, QPC * NCL], F32, tag="am")
    arg = P.tile([B, QPC], F32, tag="arg")
    idt = P.tile([B, QPC], F32, tag="idt")

    scores = P.tile([BPC, NQP], F32, tag="scores")  # per-sample all-query scores
    feat = P.tile([128, NQP * 6], F32, tag="feat")  # gather source [p, q, 6]
    tv = P.tile([BPC, NROUND * 8], F32, tag="tv")   # topk values (desc)
    ti = P.tile([BPC, NROUND * 8], dt.uint32, tag="ti")
    ti16 = P.tile([BPC, NIDX], dt.int16, tag="ti16")
    wrap = P.tile([128, NIDX // 16], dt.int16, tag="wrap")
    gout = P.tile([128, NIDX * 6], F32, tag="gout")

    # pooled results accumulate here: [4 (i rows), (b, c, f=t*20+j)]; row b's
    # 300-float block is contiguous so one 2D DMA ships it to agin[b]
    pcall = P.tile([4, BPC * 300], F32, tag="pcall")

    # ---------------- phase 1: pooling (x -> agin [8,1200] in DRAM) --------
    with tc.tile_pool(name="xp", bufs=8) as XP, \
         tc.tile_pool(name="s1p", bufs=8) as S1P, \
         tc.tile_pool(name="pps", bufs=6, space="PSUM") as PPS:
        for cx in (2, 1, 0):  # BGR->RGB handled via destination offset
            for b in range(BPC):
                # the image's two slices ride different queues, so they
                # transfer in parallel and the reduce is fed sooner
                xsrc = x_d[b, cx]  # [128, 5, 640], host-packed partition-major
                xa1 = XP.tile([128, 2 * WIMG], dt.uint8, tag="xa1", name="xa1")
                xa2 = XP.tile([128, 3 * WIMG], dt.uint8, tag="xa2", name="xa2")
                e1, e2 = (
                    (nc.sync, nc.scalar)
                    if (cx * BPC + b) % 2 == 0
                    else (nc.scalar, nc.sync)
                )
                e1.dma_start(
                    xa1[:].rearrange("p (t w) -> p t w", t=2), xsrc[:, 0:2, :]
                )
                e2.dma_start(
                    xa2[:].rearrange("p (t w) -> p t w", t=3), xsrc[:, 2:5, :]
                )
                if cx == 2 and b == 0:
                    # consts ride behind the first image (needed from ~t=20)
                    nc.scalar.dma_start(g4[:], g4_d[:])
                    nc.scalar.dma_start(id64[:], id64_d[:])
                    nc.scalar.dma_start(iod[:], iod_d[:])
                if cx == 1 and b == 0:
                    # W-gate: tiny vector copies into every wt chunk region,
                    # READING image 8's pooled block (so the scheduler cannot
                    # hoist them). The W chunk DMAs pick up a WAW dep on
                    # these writes: their transfers can't start until pooling
                    # is ~1/3 done, keeping the early DMA rings clear for x.
                    for k in range(5):
                        for c in range(3):
                            nc.vector.tensor_copy(
                                wt[k][0:1, c * CW : c * CW + 4],
                                pcall[0:1, 100:104],
                            )
                if cx == 1 and b == 5:
                    # second half of W gated later still (image 13)
                    for k in range(5, KT):
                        for c in range(3):
                            nc.vector.tensor_copy(
                                wt[k][0:1, c * CW : c * CW + 4],
                                pcall[0:1, 1600:1604],
                            )
                # width pooling: sum groups of 32 -> [128, 5*20], f32 out
                # (sums <= 8160 are exact in f32)
                s1f = S1P.tile([128, 100], F32, tag="s1f", name="s1f")
                with nc.allow_low_precision(reason="f32 sums of uint8 are exact"):
                    nc.vector.tensor_reduce(
                        s1f[:, 0:40],
                        xa1[:].rearrange("p (t j g) -> p t j g", t=2, j=20),
                        axis=AX.X,
                        op=OP.add,
                    )
                    nc.vector.tensor_reduce(
                        s1f[:, 40:100],
                        xa2[:].rearrange("p (t j g) -> p t j g", t=3, j=20),
                        axis=AX.X,
                        op=OP.add,
                    )
                # height pooling via PE: G4.T @ s1f -> [4, 100] (scaled)
                ps = PPS.tile([4, 100], F32, tag="ps", name="ps")
                nc.tensor.matmul(ps[:], g4[:], s1f[:], start=True, stop=True)
                # k-order is (i, c, t, j): pc row i -> contiguous 100-block
                base = b * 300 + (2 - cx) * 100
                nc.vector.tensor_copy(pcall[:, base : base + 100], ps[:])


    nc.vector.memset(ti16[:, :], 0)
    nc.vector.memset(feat[:, :], 0)  # only partitions 16b hold real data

    # W fetch: 30 chunks alternating sync/scalar behind the gates, in GEMM
    # consumption order (column-chunk outer) so late arrivals only affect
    # the last psum chunks
    for c in range(3):
        for k in range(KT):
            eng = nc.sync if (c * KT + k) % 2 == 0 else nc.scalar
            eng.dma_start(wt[k][:, c * CW : (c + 1) * CW], w_d[k, c])

    # ship pooled rows to the AllGather input. These queue entries block on
    # the vector pcall copies, but the W entries ahead of them have already
    # handed their transfers to the rings. Row b releases as soon as vector
    # finishes sample b's last channel copy.
    for b in range(BPC):
        eng = nc.sync if b % 2 == 0 else nc.scalar
        eng.dma_start(
            agin[b :: BPC].rearrange("j (i f) -> i j f", i=4),
            pcall[:, b * 300 : (b + 1) * 300]
            .unsqueeze(1)
            .broadcast_to((4, NCORES, 300)),
        )

    # ---------------- phase 2: feature AllGather -> yA, transposes --------
    nc.gpsimd.collective_compute(
        "AllToAll",
        mybir.AluOpType.bypass,
        replica_groups=[list(range(NCORES))],
        ins=[agin[:].opt()],
        outs=[agout[:].opt()],
    )
    # gpsimd is done with standard-library ops (pooling reduces); switch to
    # the gather library now so the ~10us reload hides under the GEMM
    nc.gpsimd.load_library(library_config.ap_gather)
    nc.sync.dma_start(yA[:], agout[:])
    with tc.tile_pool(name="pts", bufs=2, space="PSUM") as PTS:
        for k in range(KT):
            pst = PTS.tile([KTS, B], F32, tag="pst", name="pst")
            nc.tensor.transpose(pst[:], yA[:, k * KTS : (k + 1) * KTS], id64[:])
            nc.vector.tensor_copy(ptA[k][:], pst[:])

    # ---------------- phase 3: stage-2 GEMM + per-chunk postproc ----------
    with tc.tile_pool(name="ycp", bufs=4) as YCP, \
         tc.tile_pool(name="yps", bufs=6, space="PSUM") as YPS:
        for n in range(NCH2):
            psy = YPS.tile([B, NCHUNK], F32, tag="psy", name="psy")
            for k in range(KT):
                nc.tensor.matmul(
                    psy[:],
                    ptA[k][:],
                    wt[k][:, n * NCHUNK : (n + 1) * NCHUNK],
                    start=(k == 0),
                    stop=(k == KT - 1),
                )
            yc = YCP.tile([B, NCHUNK], F32, tag="yc", name="yc")
            nc.vector.tensor_copy(yc[:], psy[:])
            ycv = yc[:].rearrange("b (q c) -> b q c", q=QPC)
            av = a2as[:, n * QPC * 6 : (n + 1) * QPC * 6].rearrange(
                "b (q c) -> b q c", c=6
            )
            # boxes
            nc.vector.tensor_copy(av[:, :, 0:4], ycv[:, :, 0:4])
            # per-query max score
            nc.vector.tensor_reduce(
                mx[:, :], ycv[:, :, 4:NCHAN], axis=AX.X, op=OP.max
            )
            nc.vector.tensor_copy(av[:, :, 4], mx[:, :])
            # duplicate scores into the contiguous trailer block (so the
            # receive side can fetch all scores in one contiguous-row DMA)
            nc.vector.tensor_copy(
                a2as[:, QSH * 6 + n * QPC : QSH * 6 + (n + 1) * QPC], mx[:, :]
            )
            # argmax id (first-index ties): is_ge mask * (79 - idx), max
            eqv = eq[:].rearrange("b (q c) -> b q c", q=QPC)
            nc.vector.tensor_tensor(
                eqv,
                ycv[:, :, 4:NCHAN],
                mx[:, :].unsqueeze(-1).broadcast_to((B, QPC, NCL)),
                op=OP.is_ge,
            )
            amv = am[:].rearrange("b (q c) -> b q c", q=QPC)
            nc.vector.tensor_tensor(
                amv,
                eqv,
                iod[:B, :].unsqueeze(1).broadcast_to((B, QPC, NCL)),
                op=OP.mult,
            )
            nc.vector.tensor_reduce(arg[:, :], amv, axis=AX.X, op=OP.max)
            nc.vector.tensor_scalar(
                idt[:, :], arg[:, :], -1.0, float(NCL - 1),
                op0=OP.mult, op1=OP.add,
            )
            nc.vector.tensor_copy(av[:, :, 5], idt[:, :])

    # ---------------- phase 4: AllToAll exchange --------------------------
    nc.sync.dma_start(a2ain[:], a2as[:])
    nc.gpsimd.collective_compute(
        "AllToAll",
        mybir.AluOpType.bypass,
        replica_groups=[list(range(NCORES))],
        ins=[a2ain[:].opt()],
        outs=[a2aout[:].opt()],
    )
    # out row (j*8+i) = sample (8*core+i) data for query shard j

    # scores [8, 336]: two parallel 3D-AP DMAs (i on partitions; j, q free)
    scv = a2aout[:, QSH * 6 :].rearrange("(j i) q -> i j q", i=BPC)
    nc.sync.dma_start(scores[:, : 4 * QSH], scv[:, 0:4])
    nc.scalar.dma_start(scores[:, 4 * QSH :], scv[:, 4:8])
    # feat partition 16i = sample i's [336, 6] records: one 2D DMA each
    aaf = a2aout[:, : QSH * 6].rearrange("(j i) f -> i j f", i=BPC)
    for i in range(BPC):
        eng = nc.sync if i % 2 == 0 else nc.scalar
        eng.dma_start(
            feat[16 * i : 16 * i + 1, :].rearrange("o (j f) -> o j f", j=NCORES),
            aaf[i : i + 1, :, :],
        )

    # ---------------- phase 5: top-150 via iterated max8 ------------------
    # destructive on `scores` (nothing else reads it afterwards)
    for r in range(NROUND):
        nc.vector.max(tv[:, 8 * r : 8 * r + 8], scores[:, :])
        nc.vector.max_index(
            ti[:, 8 * r : 8 * r + 8], tv[:, 8 * r : 8 * r + 8], scores[:, :]
        )
        if r < NROUND - 1:
            nc.vector.match_replace(
                scores[:, :], tv[:, 8 * r : 8 * r + 8], scores[:, :], NEG
            )

    nc.vector.tensor_copy(ti16[:, : NROUND * 8], ti[:, :])

    # wrap indices into per-sample [16, 10] layout (via DRAM)
    nc.scalar.dma_start(tsc[:], ti16[:])
    for b in range(BPC):
        eng = nc.sync if b % 2 == 0 else nc.scalar
        eng.dma_start(
            wrap[16 * b : 16 * b + 16, :],
            tsc[b].rearrange("(f p) -> p f", p=16),
        )

    # ---------------- phase 6: gather + output ----------------------------
    nc.gpsimd.ap_gather(
        gout[:].rearrange("p (i c) -> p i c", c=6),
        feat[:].rearrange("p (q c) -> p q c", c=6),
        wrap[:],
        channels=128,
        num_elems=NQP,
        d=6,
        num_idxs=NIDX,
    )
    for b in range(BPC):
        eng = nc.sync if b % 2 == 0 else nc.scalar
        eng.dma_start(
            out_d[b : b + 1].rearrange("o k c -> o (k c)"),
            gout[16 * b : 16 * b + 1, : TOPK * 6],
        )


def _make_consts():
    g4 = np.zeros((128, 4), np.float32)
    for i in range(4):
        g4[32 * i : 32 * (i + 1), i] = np.float32(SCALE)
    id64 = np.eye(B, dtype=np.float32)
    iod = np.broadcast_to(
        (np.float32(NCL - 1) - np.arange(NCL, dtype=np.float32))[None, :], (128, NCL)
    ).copy()
    return g4, id64, iod


_NC_CACHE = {}


def _get_nc():
    if "nc" not in _NC_CACHE:
        _NC_CACHE["nc"] = build_program()
    return _NC_CACHE["nc"]


def pack_w(W: np.ndarray) -> list[np.ndarray]:
    """[1200, 25200] -> per-core [KT, 120, 3528] shards.

    Rows permuted from k=(c, h20=t*4+i, j) to k'=(i, c, t, j); cols padded
    300->336 queries with PADVAL."""
    Wr = W.reshape(CHN, 5, 4, 20, NQ * NCHAN).transpose(2, 0, 1, 3, 4)
    Wr = np.ascontiguousarray(Wr).reshape(KDIM, NQ * NCHAN)
    Wp = np.full((KDIM, NQP * NCHAN), np.float32(PADVAL), np.float32)
    Wp[:, : NQ * NCHAN] = Wr
    shards = []
    for c in range(NCORES):
        sh = Wp[:, c * NSH : (c + 1) * NSH].reshape(KT, KTS, 3, NSH // 3)
        shards.append(np.ascontiguousarray(sh.transpose(0, 2, 1, 3)))
    return shards


def pack_x(xs: np.ndarray) -> np.ndarray:
    """[BPC, 3, 640, 640] int32 -> [BPC, 3, 128, 5, 640] uint8 partition-major."""
    return np.ascontiguousarray(
        xs.reshape(BPC, CHN, 5, 128, WIMG).transpose(0, 1, 3, 2, 4).astype(np.uint8)
    )


def make_in_maps(x: np.ndarray, W: np.ndarray) -> list[dict]:
    g4, id64, iod = _make_consts()
    shards = pack_w(W)
    in_maps = []
    for c in range(NCORES):
        in_maps.append(
            {
                "x": pack_x(x[c * BPC : (c + 1) * BPC]),
                "w": shards[c],
                "g4": g4,
                "id64": id64,
                "iod": iod,
            }
        )
    return in_maps


def kernel(x: np.ndarray, W: np.ndarray) -> np.ndarray:
    x = np.ascontiguousarray(np.asarray(x), dtype=np.int32)
    W = np.ascontiguousarray(np.asarray(W), dtype=np.float32)
    assert x.shape == (B, CHN, HIMG, WIMG) and W.shape == (KDIM, NQ * NCHAN)

    nc = _get_nc()
    in_maps = make_in_maps(x, W)
    res = run_bass_kernel_spmd(nc, in_maps, core_ids=list(range(NCORES)))
    out = np.concatenate([res.results[c]["out"] for c in range(NCORES)], axis=0)
    return out.astype(np.float32)


if __name__ == "__main__":
    xs = np.random.randint(0, 256, (B, CHN, HIMG, WIMG)).astype(np.int32)
    Ws = (np.random.randn(KDIM, NQ * NCHAN) * 0.02).astype(np.float32)
    o = kernel(xs, Ws)
    print("kernel output:", o.shape, o.dtype)


# revision 7
# speedup vs baseline: 1.2021x; 1.2021x over previous
"""Trainium2 Bass kernel v2: pool -> AllGather(features) -> N-sharded linear
-> AllToAll(per-query results) -> per-sample topk+gather.

Stage 1 (data parallel): each core pools its 8 samples' images -> [8, 1200]
features, written straight into the AllGather input buffer in DRAM.
AllGather replicates all 64 samples' features to every core (307KB).

Stage 2 (model parallel): W's output dim is padded 300->336 queries and
sharded 8 ways (42 queries = 3528 cols/core, 16.9MB instead of 121MB).
Each core computes y[64, 3528] and reduces it to per-query
[boxes(4), score, argmax-id] records.

Stage 3: AllToAll (64KB) routes each sample's 8 query-shard records to the
sample's owner core; owner runs top-150 + gather exactly like the
data-parallel baseline and writes its 8 samples' output.

Self-contained: hardcodes all shapes; builds one SPMD Bass program and runs
it via run_bass_kernel_spmd on cores 0-7.
"""

import os
import sys
from contextlib import ExitStack

import numpy as np

for _p in ("/opt/trn_rl_repo", "/root/.axon_site/_ro/trn_rl_repo"):
    if os.path.isdir(_p) and _p not in sys.path:
        sys.path.append(_p)

import concourse.bass as bass
import concourse.tile as tile
from concourse import bacc, library_config, mybir
from concourse.bass_utils import run_bass_kernel_spmd

dt = mybir.dt
F32 = dt.float32
AX = mybir.AxisListType
OP = mybir.AluOpType

# ---------------- problem constants (hardcoded) ----------------
B, CHN, HIMG, WIMG = 64, 3, 640, 640
NQ, NCHAN, NCL, TOPK = 300, 84, 80, 150
KDIM = 1200                         # 3*20*20 features
NCORES = 8
BPC = B // NCORES                   # samples per core = 8
KT, KTS = 10, 120                   # GEMM K tiling: 10 x 120
NQP = 336                           # queries padded to 8*42
QSH = NQP // NCORES                 # queries per core shard = 42
NSH = QSH * NCHAN                   # stage-2 cols per core = 3528
QPC = 6                             # queries per psum chunk
NCHUNK = QPC * NCHAN                # 504 cols per chunk (psum bank)
NCH2 = NSH // NCHUNK                # 7 chunks
PQ = QSH * 7                        # a2a payload: [42 x (box4,score,id)] + [42 scores]
SCALE = np.float64(1.0) / (32 * 32 * 255)
NEG = -3.0e38
PADVAL = -1.0                       # padded W cols: score ~ -560, never top-k
NIDX = 160                          # padded top-k index count (152 used)
NROUND = 19                         # 19 rounds x 8 = 152 >= 150


def build_program():
    nc = bacc.Bacc("TRN2", target_bir_lowering=False, debug=False,
                   num_devices=NCORES)
    # x host-packed uint8, partition-major: every DMA is a linear read
    x_d = nc.dram_tensor(
        "x", [BPC, CHN, 128, 5, WIMG], dt.uint8, kind="ExternalInput"
    )
    # per-core W shard, (k-tile, col-chunk) major so each chunk is linear
    w_d = nc.dram_tensor("w", [KT, 3, KTS, NSH // 3], F32, kind="ExternalInput")
    g4_d = nc.dram_tensor("g4", [128, 4], F32, kind="ExternalInput")
    id64_d = nc.dram_tensor("id64", [B, B], F32, kind="ExternalInput")
    iod_d = nc.dram_tensor("iod", [128, NCL], F32, kind="ExternalInput")
    out_d = nc.dram_tensor("out", [BPC, TOPK, 6], F32, kind="ExternalOutput")

    with tile.TileContext(nc) as tc:
        with ExitStack() as ctx:
            _body(ctx, tc, x_d, w_d, g4_d, id64_d, iod_d, out_d)
    nc.finalize()
    return nc


def _body(ctx, tc, x_d, w_d, g4_d, id64_d, iod_d, out_d):
    nc = tc.nc

    # ---------------- persistent tiles ----------------
    P = ctx.enter_context(tc.tile_pool(name="persist", bufs=1))
    D = ctx.enter_context(tc.tile_pool(name="dramp", bufs=1, space="DRAM"))

    g4 = P.tile([128, 4], F32, tag="g4")
    id64 = P.tile([B, B], F32, tag="id64")
    iod = P.tile([128, NCL], F32, tag="iod")

    # W shard: all 10 k-tiles resident (16.9MB SBUF). The chunk DMAs are
    # issued after the pooling loop, gated behind the last x transfer, so
    # the x feed owns the DMA rings while pooling runs.
    wt = [P.tile([KTS, NSH], F32, tag=f"wt{k}", name=f"wt{k}") for k in range(KT)]
    CW = NSH // 3

    # a tiny dummy AllReduce fired at t~0: it rendezvouses the 8 cores on
    # the CC cores while pooling runs, so the launch skew (~17us) is paid
    # here -- hidden under pooling -- instead of at the feature exchange.
    sk = D.tile([8, 4], F32, tag="sk")
    nc.gpsimd.collective_compute(
        "AllReduce",
        mybir.AluOpType.add,
        replica_groups=[list(range(NCORES))],
        ins=[sk[:].opt()],
        outs=[sk[:].opt()],
    )

    # collective bounce buffers (DRAM). The feature gather runs as an
    # AllToAll with the local block replicated 8x on the send side: block j
    # holds MY pooled rows, so core c's output block j = core j's rows --
    # the full gather -- while dodging the mesh-AllGather's ring latency.
    agin = D.tile([B, KDIM], F32, tag="agin")
    agout = D.tile([B, KDIM], F32, tag="agout")
    a2ain = D.tile([B, PQ], F32, tag="a2ain")       # records + score trailer
    a2aout = D.tile([B, PQ], F32, tag="a2aout")
    tsc = D.tile([BPC, NIDX], dt.int16, tag="tsc")  # topk idx bounce

    yA = P.tile([B, KDIM], F32, tag="yA")           # gathered features SBUF
    ptA = [P.tile([KTS, B], F32, tag=f"pt{k}", name=f"pt{k}") for k in range(KT)]
    a2as = P.tile([B, PQ], F32, tag="a2as")         # a2a send staging
    mx = P.tile([B, QPC], F32, tag="mx")
    eq = P.tile([B, QPC * NCL], F32, tag="eq")
    am = P.tile([B, QPC * NCL], F32, tag="am")
    arg = P.tile([B, QPC], F32, tag="arg")
    idt = P.tile([B, QPC], F32, tag="idt")

    scores = P.tile([BPC, NQP], F32, tag="scores")  # per-sample all-query scores
    feat = P.tile([128, NQP * 6], F32, tag="feat")  # gather source [p, q, 6]
    tv = P.tile([BPC, NROUND * 8], F32, tag="tv")   # topk values (desc)
    ti = P.tile([BPC, NROUND * 8], dt.uint32, tag="ti")
    ti16 = P.tile([BPC, NIDX], dt.int16, tag="ti16")
    wrap = P.tile([128, NIDX // 16], dt.int16, tag="wrap")
    gout = P.tile([128, NIDX * 6], F32, tag="gout")

    # pooled results accumulate here: [4 (i rows), (b, c, f=t*20+j)]; row b's
    # 300-float block is contiguous so one 2D DMA ships it to agin[b]
    pcall = P.tile([4, BPC * 300], F32, tag="pcall")

    # ---------------- phase 1: pooling (x -> agin [8,1200] in DRAM) --------
    with tc.tile_pool(name="xp", bufs=8) as XP, \
         tc.tile_pool(name="s1p", bufs=6) as S1P, \
         tc.tile_pool(name="pps", bufs=4, space="PSUM") as PPS:
        for cx in (2, 1, 0):  # BGR->RGB handled via destination offset
            for b in range(BPC):
                # the image's two slices ride different queues, so they
                # transfer in parallel and the reduce is fed sooner
                xsrc = x_d[b, cx]  # [128, 5, 640], host-packed partition-major
                xa1 = XP.tile([128, 2 * WIMG], dt.uint8, tag="xa1", name="xa1")
                xa2 = XP.tile([128, 3 * WIMG], dt.uint8, tag="xa2", name="xa2")
                nc.sync.dma_start(
                    xa1[:].rearrange("p (t w) -> p t w", t=2), xsrc[:, 0:2, :]
                )
                nc.scalar.dma_start(
                    xa2[:].rearrange("p (t w) -> p t w", t=3), xsrc[:, 2:5, :]
                )
                if cx == 2 and b == 0:
                    # consts ride behind the first image (needed from ~t=20)
                    nc.scalar.dma_start(g4[:], g4_d[:])
                    nc.scalar.dma_start(id64[:], id64_d[:])
                    nc.scalar.dma_start(iod[:], iod_d[:])
                if cx == 1 and b == 0:
                    # W-gate: tiny vector copies into every wt chunk region,
                    # READING image 8's pooled block (so the scheduler cannot
                    # hoist them). The W chunk DMAs pick up a WAW dep on
                    # these writes: their transfers can't start until pooling
                    # is ~1/3 done, keeping the early DMA rings clear for x.
                    for k in range(5):
                        for c in range(3):
                            nc.vector.tensor_copy(
                                wt[k][0:1, c * CW : c * CW + 4],
                                pcall[0:1, 100:104],
                            )
                if cx == 1 and b == 5:
                    # second half of W gated later still (image 13)
                    for k in range(5, KT):
                        for c in range(3):
                            nc.vector.tensor_copy(
                                wt[k][0:1, c * CW : c * CW + 4],
                                pcall[0:1, 1600:1604],
                            )
                # width pooling: sum groups of 32 -> [128, 5*20], f32 out
                # (sums <= 8160 are exact in f32)
                s1f = S1P.tile([128, 100], F32, tag="s1f", name="s1f")
                with nc.allow_low_precision(reason="f32 sums of uint8 are exact"):
                    nc.vector.tensor_reduce(
                        s1f[:, 0:40],
                        xa1[:].rearrange("p (t j g) -> p t j g", t=2, j=20),
                        axis=AX.X,
                        op=OP.add,
                    )
                    nc.vector.tensor_reduce(
                        s1f[:, 40:100],
                        xa2[:].rearrange("p (t j g) -> p t j g", t=3, j=20),
                        axis=AX.X,
                        op=OP.add,
                    )
                # height pooling via PE: G4.T @ s1f -> [4, 100] (scaled)
                ps = PPS.tile([4, 100], F32, tag="ps", name="ps")
                nc.tensor.matmul(ps[:], g4[:], s1f[:], start=True, stop=True)
                # k-order is (i, c, t, j): pc row i -> contiguous 100-block
                base = b * 300 + (2 - cx) * 100
                nc.vector.tensor_copy(pcall[:, base : base + 100], ps[:])


    nc.vector.memset(ti16[:, :], 0)
    nc.vector.memset(feat[:, :], 0)  # only partitions 16b hold real data

    # W fetch: 30 chunks alternating sync/scalar behind the gates, in GEMM
    # consumption order (column-chunk outer) so late arrivals only affect
    # the last psum chunks
    for c in range(3):
        for k in range(KT):
            eng = nc.sync if (c * KT + k) % 2 == 0 else nc.scalar
            eng.dma_start(wt[k][:, c * CW : (c + 1) * CW], w_d[k, c])

    # ship pooled rows to the AllGather input. These queue entries block on
    # the vector pcall copies, but the W entries ahead of them have already
    # handed their transfers to the rings. Row b releases as soon as vector
    # finishes sample b's last channel copy.
    for b in range(BPC):
        eng = nc.sync if b % 2 == 0 else nc.scalar
        eng.dma_start(
            agin[b :: BPC].rearrange("j (i f) -> i j f", i=4),
            pcall[:, b * 300 : (b + 1) * 300]
            .unsqueeze(1)
            .broadcast_to((4, NCORES, 300)),
        )

    # ---------------- phase 2: feature AllGather -> yA, transposes --------
    nc.gpsimd.collective_compute(
        "AllToAll",
        mybir.AluOpType.bypass,
        replica_groups=[list(range(NCORES))],
        ins=[agin[:].opt()],
        outs=[agout[:].opt()],
    )
    # gpsimd is done with standard-library ops (pooling reduces); switch to
    # the gather library now so the ~10us reload hides under the GEMM
    nc.gpsimd.load_library(library_config.ap_gather)
    nc.sync.dma_start(yA[:], agout[:])
    with tc.tile_pool(name="pts", bufs=2, space="PSUM") as PTS:
        for k in range(KT):
            pst = PTS.tile([KTS, B], F32, tag="pst", name="pst")
            nc.tensor.transpose(pst[:], yA[:, k * KTS : (k + 1) * KTS], id64[:])
            nc.vector.tensor_copy(ptA[k][:], pst[:])

    # ---------------- phase 3: stage-2 GEMM + per-chunk postproc ----------
    with tc.tile_pool(name="ycp", bufs=4) as YCP, \
         tc.tile_pool(name="yps", bufs=6, space="PSUM") as YPS:
        for n in range(NCH2):
            psy = YPS.tile([B, NCHUNK], F32, tag="psy", name="psy")
            for k in range(KT):
                nc.tensor.matmul(
                    psy[:],
                    ptA[k][:],
                    wt[k][:, n * NCHUNK : (n + 1) * NCHUNK],
                    start=(k == 0),
                    stop=(k == KT - 1),
                )
            yc = YCP.tile([B, NCHUNK], F32, tag="yc", name="yc")
            nc.vector.tensor_copy(yc[:], psy[:])
            ycv = yc[:].rearrange("b (q c) -> b q c", q=QPC)
            av = a2as[:, n * QPC * 6 : (n + 1) * QPC * 6].rearrange(
                "b (q c) -> b q c", c=6
            )
            # boxes
            nc.vector.tensor_copy(av[:, :, 0:4], ycv[:, :, 0:4])
            # per-query max score
            nc.vector.tensor_reduce(
                mx[:, :], ycv[:, :, 4:NCHAN], axis=AX.X, op=OP.max
            )
            nc.vector.tensor_copy(av[:, :, 4], mx[:, :])
            # duplicate scores into the contiguous trailer block (so the
            # receive side can fetch all scores in one contiguous-row DMA)
            nc.vector.tensor_copy(
                a2as[:, QSH * 6 + n * QPC : QSH * 6 + (n + 1) * QPC], mx[:, :]
            )
            # argmax id (first-index ties): is_ge mask * (79 - idx), max
            eqv = eq[:].rearrange("b (q c) -> b q c", q=QPC)
            nc.vector.tensor_tensor(
                eqv,
                ycv[:, :, 4:NCHAN],
                mx[:, :].unsqueeze(-1).broadcast_to((B, QPC, NCL)),
                op=OP.is_ge,
            )
            amv = am[:].rearrange("b (q c) -> b q c", q=QPC)
            nc.vector.tensor_tensor(
                amv,
                eqv,
                iod[:B, :].unsqueeze(1).broadcast_to((B, QPC, NCL)),
                op=OP.mult,
            )
            nc.vector.tensor_reduce(arg[:, :], amv, axis=AX.X, op=OP.max)
            nc.vector.tensor_scalar(
                idt[:, :], arg[:, :], -1.0, float(NCL - 1),
                op0=OP.mult, op1=OP.add,
            )
            nc.vector.tensor_copy(av[:, :, 5], idt[:, :])

    # ---------------- phase 4: AllToAll exchange --------------------------
    nc.sync.dma_start(a2ain[:], a2as[:])
    nc.gpsimd.collective_compute(
        "AllToAll",
        mybir.AluOpType.bypass,
        replica_groups=[list(range(NCORES))],
        ins=[a2ain[:].opt()],
        outs=[a2aout[:].opt()],
    )
    # out row (j*8+i) = sample (8*core+i) data for query shard j

    # scores [8, 336]: two parallel 3D-AP DMAs (i on partitions; j, q free)
    scv = a2aout[:, QSH * 6 :].rearrange("(j i) q -> i j q", i=BPC)
    nc.sync.dma_start(scores[:, : 4 * QSH], scv[:, 0:4])
    nc.scalar.dma_start(scores[:, 4 * QSH :], scv[:, 4:8])
    # feat partition 16i = sample i's [336, 6] records: one 2D DMA each
    aaf = a2aout[:, : QSH * 6].rearrange("(j i) f -> i j f", i=BPC)
    for i in range(BPC):
        eng = nc.sync if i % 2 == 0 else nc.scalar
        eng.dma_start(
            feat[16 * i : 16 * i + 1, :].rearrange("o (j f) -> o j f", j=NCORES),
            aaf[i : i + 1, :, :],
        )

    # ---------------- phase 5: top-150 via iterated max8 ------------------
    # destructive on `scores` (nothing else reads it afterwards)
    for r in range(NROUND):
        nc.vector.max(tv[:, 8 * r : 8 * r + 8], scores[:, :])
        nc.vector.max_index(
            ti[:, 8 * r : 8 * r + 8], tv[:, 8 * r : 8 * r + 8], scores[:, :]
        )
        if r < NROUND - 1:
            nc.vector.match_replace(
                scores[:, :], tv[:, 8 * r : 8 * r + 8], scores[:, :], NEG
            )

    nc.vector.tensor_copy(ti16[:, : NROUND * 8], ti[:, :])

    # wrap indices into per-sample [16, 10] layout (via DRAM)
    nc.scalar.dma_start(tsc[:], ti16[:])
    for b in range(BPC):
        eng = nc.sync if b % 2 == 0 else nc.scalar
        eng.dma_start(
            wrap[16 * b : 16 * b + 16, :],
            tsc[b].rearrange("(f p) -> p f", p=16),
        )

    # ---------------- phase 6: gather + output ----------------------------
    nc.gpsimd.ap_gather(
        gout[:].rearrange("p (i c) -> p i c", c=6),
        feat[:].rearrange("p (q c) -> p q c", c=6),
        wrap[:],
        channels=128,
        num_elems=NQP,
        d=6,
        num_idxs=NIDX,
    )
    for b in range(BPC):
        eng = nc.sync if b % 2 == 0 else nc.scalar
        eng.dma_start(
            out_d[b : b + 1].rearrange("o k c -> o (k c)"),
            gout[16 * b : 16 * b + 1, : TOPK * 6],
        )


def _make_consts():
    g4 = np.zeros((128, 4), np.float32)
    for i in range(4):
        g4[32 * i : 32 * (i + 1), i] = np.float32(SCALE)
    id64 = np.eye(B, dtype=np.float32)
    iod = np.broadcast_to(
        (np.float32(NCL - 1) - np.arange(NCL, dtype=np.float32))[None, :], (128, NCL)
    ).copy()
    return g4, id64, iod


_NC_CACHE = {}


def _get_nc():
    if "nc" not in _NC_CACHE:
        _NC_CACHE["nc"] = build_program()
    return _NC_CACHE["nc"]


def pack_w(W: np.ndarray) -> list[np.ndarray]:
    """[1200, 25200] -> per-core [KT, 120, 3528] shards.

    Rows permuted from k=(c, h20=t*4+i, j) to k'=(i, c, t, j); cols padded
    300->336 queries with PADVAL."""
    Wr = W.reshape(CHN, 5, 4, 20, NQ * NCHAN).transpose(2, 0, 1, 3, 4)
    Wr = np.ascontiguousarray(Wr).reshape(KDIM, NQ * NCHAN)
    Wp = np.full((KDIM, NQP * NCHAN), np.float32(PADVAL), np.float32)
    Wp[:, : NQ * NCHAN] = Wr
    shards = []
    for c in range(NCORES):
        sh = Wp[:, c * NSH : (c + 1) * NSH].reshape(KT, KTS, 3, NSH // 3)
        shards.append(np.ascontiguousarray(sh.transpose(0, 2, 1, 3)))
    return shards


def pack_x(xs: np.ndarray) -> np.ndarray:
    """[BPC, 3, 640, 640] int32 -> [BPC, 3, 128, 5, 640] uint8 partition-major."""
    return np.ascontiguousarray(
        xs.reshape(BPC, CHN, 5, 128, WIMG).transpose(0, 1, 3, 2, 4).astype(np.uint8)
    )


def make_in_maps(x: np.ndarray, W: np.ndarray) -> list[dict]:
    g4, id64, iod = _make_consts()
    shards = pack_w(W)
    in_maps = []
    for c in range(NCORES):
        in_maps.append(
            {
                "x": pack_x(x[c * BPC : (c + 1) * BPC]),
                "w": shards[c],
                "g4": g4,
                "id64": id64,
                "iod": iod,
            }
        )
    return in_maps


def kernel(x: np.ndarray, W: np.ndarray) -> np.ndarray:
    x = np.ascontiguousarray(np.asarray(x), dtype=np.int32)
    W = np.ascontiguousarray(np.asarray(W), dtype=np.float32)
    assert x.shape == (B, CHN, HIMG, WIMG) and W.shape == (KDIM, NQ * NCHAN)

    nc = _get_nc()
    in_maps = make_in_maps(x, W)
    res = run_bass_kernel_spmd(nc, in_maps, core_ids=list(range(NCORES)))
    out = np.concatenate([res.results[c]["out"] for c in range(NCORES)], axis=0)
    return out.astype(np.float32)


if __name__ == "__main__":
    xs = np.random.randint(0, 256, (B, CHN, HIMG, WIMG)).astype(np.int32)
    Ws = (np.random.randn(KDIM, NQ * NCHAN) * 0.02).astype(np.float32)
    o = kernel(xs, Ws)
    print("kernel output:", o.shape, o.dtype)
